# revision 1
# baseline (speedup 1.0000x reference)
"""Trainium2 Bass kernel for nn_AttentionReadout (sparse cross-attention + FiLM readout).

Sharding: 8 cores = 4 batches x 2 query-halves. Each core owns Q=S/2 queries of one
batch. The r2l/reversal symmetry maps the second query-half onto the first-half
program (streams swapped + token-reversed), so one SPMD program serves all cores
with static causal tile-skipping. The temb-MLP/FiLM path (tiny FLOPs, heavy weight
traffic) is sharded 8-way over hidden dims with AllGather exchanges.

On-chip layout is feature-major [features(partition), tokens(free)]: weights [in,out]
serve directly as matmul lhsT. LayerNorm per-token stats use ones-matmuls (partition
reduction) and K=1 ones-matmul broadcasts; LN affines are folded into the following
projection weights host-side.
"""

import sys

for _p in ("/opt/trn_rl_repo", "/opt/pypackages"):
    if _p not in sys.path:
        sys.path.insert(0, _p)

from contextlib import ExitStack

import numpy as np

import concourse.bass as bass
import concourse.mybir as mybir
import concourse.tile as tile
from concourse import bacc

F32 = mybir.dt.float32
BF16 = mybir.dt.bfloat16
F32R = mybir.dt.float32r
AF = mybir.ActivationFunctionType
ALU = mybir.AluOpType

EPS = 1e-5


def _r(ap):
    return ap


class Cfg:
    def __init__(self, B, S, E, H, D1, S_OUT, L, NC=8):
        self.B, self.S, self.E, self.H = B, S, E, H
        self.HD = E // H
        self.D1 = D1          # mlp hidden (res block hidden too)
        self.D4 = 4 * E       # film dim
        self.SO = S_OUT
        self.L = L
        self.NC = NC
        self.Q = S // 2                 # queries per core
        self.E2 = 2 * E
        self.nE = E // 128
        self.n2E = self.E2 // 128
        self.nD1 = D1 // 128
        self.nD4 = self.D4 // 128
        self.nSO = (S_OUT + 127) // 128
        self.nq = self.Q // 128         # query tiles per core
        self.nS1 = S // 128             # stream-1 kv tiles
        self.SH1 = D1 // NC             # film shard sizes
        self.SH4 = self.D4 // NC
        assert self.HD == 64
        assert self.Q % 128 == 0 and S % 128 == 0 and E % 128 == 0
        assert D1 % 128 == 0 and self.D4 % 128 == 0 and self.SH4 % 128 == 0


REAL = Cfg(B=4, S=1024, E=768, H=12, D1=3072, S_OUT=256, L=2)
MINI = Cfg(B=4, S=256, E=256, H=4, D1=512, S_OUT=128, L=2)


# ---------------------------------------------------------------- host prep

def host_prep(cfg, inp):
    """Returns per-core input maps (list of dicts of np.float32 arrays)."""
    import ml_dtypes
    c32 = lambda a: np.ascontiguousarray(a, dtype=np.float32)
    cbf = lambda a: np.ascontiguousarray(np.asarray(a, dtype=np.float32),
                                         dtype=ml_dtypes.bfloat16)
    E, Q, L = cfg.E, cfg.Q, cfg.L
    E2 = cfg.E2
    Wq, Wk, Wv = np.asarray(inp["Wq"]), np.asarray(inp["Wk"]), np.asarray(inp["Wv"])
    bk, bv = np.asarray(inp["bk"]), np.asarray(inp["bv"])
    scale = 1.0 / np.sqrt(np.float32(cfg.HD))

    # film folds: absorb res_ln affine into film coefficients
    film_w = np.array(inp["film_w"], dtype=np.float64)
    film_b = np.array(inp["film_b"], dtype=np.float64)
    fw = film_w.copy()
    fb = film_b.copy()
    for l in range(L):
        g = np.asarray(inp["res_ln_g"][l], dtype=np.float64)
        bb = np.asarray(inp["res_ln_b"][l], dtype=np.float64)
        fw[l][:, E2:] = film_w[l][:, E2:] + film_w[l][:, :E2] * bb[None, :]
        fb[l][E2:] = film_b[l][E2:] + film_b[l][:E2] * bb
        fw[l][:, :E2] = film_w[l][:, :E2] * g[None, :]
        fb[l][:E2] = film_b[l][:E2] * g

    def pack_p(w, cast):
        # like pack but pads M up to a multiple of 128 first
        w = np.asarray(w, dtype=np.float32)
        K, M = w.shape
        Mp = ((M + 127) // 128) * 128
        if Mp != M:
            w = np.concatenate([w, np.zeros((K, Mp - M), np.float32)], axis=1)
        return pack(w, cast)

    c32p = lambda a: np.ascontiguousarray(a, dtype=np.float32)

    def pack(w, cast):
        # W [K, M] -> [nM, 128(p), nK, 128(c)]: packed[m,p,k,c] = W[k*128+p, m*128+c]
        w = np.asarray(w, dtype=np.float32)
        K, M = w.shape
        nK, nM = K // 128, M // 128
        t = w.reshape(nK, 128, nM, 128).transpose(2, 1, 0, 3)
        return cast(t)

    maps = []
    for c in range(cfg.NC):
        b, hq = c // 2, c % 2
        rev = hq == 1
        l2r = np.asarray(inp["l2r_embed"][b])
        r2l = np.asarray(inp["r2l_embed"][b])
        if rev:
            s1, s2 = r2l[::-1], l2r[::-1]
            g1, bl1 = np.asarray(inp["ln2_g"]), np.asarray(inp["ln2_b"])
            g2, bl2 = np.asarray(inp["ln1_g"]), np.asarray(inp["ln1_b"])
        else:
            s1, s2 = l2r, r2l
            g1, bl1 = np.asarray(inp["ln1_g"]), np.asarray(inp["ln1_b"])
            g2, bl2 = np.asarray(inp["ln2_g"]), np.asarray(inp["ln2_b"])
        sh1 = slice(c * cfg.SH1, (c + 1) * cfg.SH1)
        sh4 = slice(c * cfg.SH4, (c + 1) * cfg.SH4)
        onehot = np.zeros(cfg.B, np.float32)
        onehot[b] = 1.0
        i128 = np.arange(128)
        m = {
            "s1T": c32(s1.T),                       # [E, S]
            "s2T": c32(s2[:Q].T),                   # [E, Q]
            "tembT": cbf(np.asarray(inp["temb"]).T),    # [E, B]
            "temb_own": cbf(np.asarray(inp["temb"])[b : b + 1].T),  # [E, 1]
            "wq1": pack(g1[:, None] * Wq * scale, cbf),
            "wq2": pack(g2[:, None] * Wq * scale, cbf),
            "qb": c32((bl1 + bl2) @ Wq * scale),
            "wk1": pack(g1[:, None] * Wk, cbf),
            "kb1": c32(bl1 @ Wk + bk),
            "wk2": pack(g2[:, None] * Wk, cbf),
            "kb2": c32(bl2 @ Wk + bk),
            "wkt": pack(Wk, cbf),
            "kbt": c32(bk),
            "wv1": cbf(g1[:, None] * Wv),
            "vb1": c32(bl1 @ Wv + bv),
            "wv2": cbf(g2[:, None] * Wv),
            "vb2": c32(bl2 @ Wv + bv),
            "wvt": cbf(Wv),
            "vbt": c32(bv),
            "wo": pack(inp["Wo"], cbf),
            "ob": c32(inp["bo"]),
            "inw": pack(inp["in_w"], cbf),
            "inb": c32(inp["in_b"]),
            "rw1": np.stack([pack(np.asarray(inp["res_w1"])[l], cbf) for l in range(L)]),
            "rb1": c32(inp["res_b1"]),
            "rw2": np.stack([pack(np.asarray(inp["res_w2"])[l], cbf) for l in range(L)]),
            "rb2": c32(inp["res_b2"]),
            "logw": pack(inp["log_w"], cbf),
            "logb": c32(inp["log_b"]),
            "mw1s": pack_p(np.asarray(inp["mlp_w1"])[:, sh1], cbf),
            "mb1s": c32(np.asarray(inp["mlp_b1"])[sh1]),
            "mw2s": pack_p(np.asarray(inp["mlp_w2"])[:, sh4], cbf),
            "mb2s": c32(np.asarray(inp["mlp_b2"])[sh4]),
            "fws": np.stack([pack_p(fw[l][:, c * cfg.SH4 : (c + 1) * cfg.SH4], cbf) for l in range(L)]),
            "fbs": c32(fb[:, sh4]),
            "onehot": cbf(onehot),
            "mask_s1": cbf(i128[:, None] > i128[None, :]),  # keep kv > q
            "mask_s2": cbf(i128[:, None] < i128[None, :]),  # keep kv < q
        }
        maps.append(m)
    return maps


def host_gather(cfg, results):
    out = np.zeros((cfg.B, cfg.S, cfg.SO), np.float32)
    for c in range(cfg.NC):
        b, hq = c // 2, c % 2
        o = np.asarray(results[c]["outT"]).T  # [Q, SO]
        if hq == 1:
            o = o[::-1]
        out[b, hq * cfg.Q : (hq + 1) * cfg.Q] = o
    return out


# ---------------------------------------------------------------- device program

def build_nc(cfg):
    nc = bacc.Bacc(None, target_bir_lowering=False, debug=True)
    B, E, Q, S, H, HD, L = cfg.B, cfg.E, cfg.Q, cfg.S, cfg.H, cfg.HD, cfg.L
    nE, n2E, nD1, nD4, nSO, nq, nS1 = (
        cfg.nE, cfg.n2E, cfg.nD1, cfg.nD4, cfg.nSO, cfg.nq, cfg.nS1)
    E2, D1, D4, SO, NC, SH1, SH4 = (
        cfg.E2, cfg.D1, cfg.D4, cfg.SO, cfg.NC, cfg.SH1, cfg.SH4)
    KVT = nS1 + nq  # kv tiles (s1 then s2), temb handled separately
    TB = min(512, Q)

    P = {}
    def di(name, shape, dt=F32):
        P[name] = nc.declare_dram_parameter(name, list(shape), dt, isOutput=False)
        return P[name]

    nms1 = (SH1 + 127) // 128
    nms4 = SH4 // 128
    di("s1T", [E, S]); di("s2T", [E, Q]); di("tembT", [E, B], BF16); di("temb_own", [E, 1], BF16)
    for w in ("wq1", "wq2", "wk1", "wk2", "wkt", "wo"):
        di(w, [nE, 128, nE, 128], BF16)
    for w in ("wv1", "wv2", "wvt"):
        di(w, [E, E], BF16)
    for v in ("qb", "kb1", "kb2", "kbt", "vb1", "vb2", "vbt", "ob"):
        di(v, [E])
    di("inw", [n2E, 128, nE, 128], BF16); di("inb", [E2])
    di("rw1", [L, nD1, 128, n2E, 128], BF16); di("rb1", [L, D1])
    di("rw2", [L, n2E, 128, nD1, 128], BF16); di("rb2", [L, E2])
    di("logw", [nSO, 128, n2E, 128], BF16); di("logb", [SO])
    di("mw1s", [nms1, 128, nE, 128], BF16); di("mb1s", [SH1])
    di("mw2s", [nms4, 128, nD1, 128], BF16); di("mb2s", [SH4])
    di("fws", [L, nms4, 128, nD4, 128], BF16); di("fbs", [L, SH4])
    di("onehot", [B], BF16); di("mask_s1", [128, 128], BF16); di("mask_s2", [128, 128], BF16)
    outT = nc.declare_dram_parameter("outT", [SO, Q], F32, isOutput=True)

    h_sh = nc.dram_tensor("h_sh", [SH1, B], BF16)
    h_g = nc.dram_tensor("h_g", [NC, SH1, B], BF16, addr_space="Shared")
    t_sh = nc.dram_tensor("t_sh", [SH4, B], BF16)
    t_g = nc.dram_tensor("t_g", [NC, SH4, B], BF16, addr_space="Shared")
    f_sh = nc.dram_tensor("f_sh", [L, SH4, B], BF16)
    f_g = nc.dram_tensor("f_g", [NC, L, SH4, B], BF16, addr_space="Shared")

    with tile.TileContext(nc) as tc, ExitStack() as stk:
        ec = stk.enter_context
        dma = nc.sync.dma_start

        # ---------------- constants (persist whole program)
        singles = ec(tc.tile_pool(name="singles", bufs=1))
        def single(shape, tagn, dt=F32):
            return singles.tile(shape, dt, name=tagn, tag=tagn)

        inv_E = single([128, 1], "inv_E", BF16); nc.vector.memset(inv_E, 1.0 / E)
        inv_2E = single([128, 1], "inv_2E", BF16); nc.vector.memset(inv_2E, 1.0 / E2)
        ones_row = single([1, 128], "ones_row", BF16); nc.vector.memset(ones_row, 1.0)
        eps_row = single([1, 1], "eps_row"); nc.vector.memset(eps_row, EPS)
        m_s1 = single([128, 128], "m_s1", BF16); dma(out=m_s1, in_=P["mask_s1"][:, :])
        m_s2 = single([128, 128], "m_s2", BF16); dma(out=m_s2, in_=P["mask_s2"][:, :])
        oh_b = single([128, B], "oh_b", BF16)
        nc.gpsimd.dma_start(out=oh_b, in_=P["onehot"][:].rearrange("(o b) -> o b", o=1).to_broadcast((128, B)))
        vb_bc = {}
        for v in ("vb1", "vb2", "vbt"):
            t = single([128, E], f"bc_{v}")
            dma(out=t, in_=P[v][:].rearrange("(o e) -> o e", o=1).to_broadcast((128, E)))
            vb_bc[v] = t
        bcols = {}
        for v, n in (("qb", nE), ("kb1", nE), ("kb2", nE), ("kbt", nE), ("ob", nE),
                     ("inb", n2E)):
            t = single([128, n], f"col_{v}")
            dma(out=t, in_=P[v][:].rearrange("(m p) -> p m", p=128))
            bcols[v] = t
        lbc = single([128, nSO], "col_logb")
        if SO % 128 == 0:
            dma(out=lbc, in_=P["logb"][:].rearrange("(m p) -> p m", p=128))
        else:
            dma(out=lbc[:SO, 0:1], in_=P["logb"][:].rearrange("(s o) -> s o", o=1))
        rb1c, rb2c = [], []
        for l in range(L):
            t1 = single([128, nD1], f"rb1c{l}")
            dma(out=t1, in_=P["rb1"][l].rearrange("(m p) -> p m", p=128))
            rb1c.append(t1)
            t2 = single([128, n2E], f"rb2c{l}")
            dma(out=t2, in_=P["rb2"][l].rearrange("(m p) -> p m", p=128))
            rb2c.append(t2)
        acol = [single([128, n2E], f"acol{l}") for l in range(L)]
        b2col = [single([128, n2E], f"b2col{l}") for l in range(L)]
        tembT_sb = single([128, nE, B], "tembT_sb", BF16)
        nc.gpsimd.dma_start(out=tembT_sb, in_=P["tembT"][:, :].rearrange("(k p) b -> p k b", p=128))
        temb_own_sb = single([128, nE], "temb_own_sb", BF16)
        nc.gpsimd.dma_start(out=temb_own_sb, in_=P["temb_own"][:, :].rearrange("(k p) o -> p (k o)", p=128))
        mb1c = single([128, nms1], "mb1c")
        if SH1 % 128 == 0:
            nc.gpsimd.dma_start(out=mb1c, in_=P["mb1s"][:].rearrange("(m p) -> p m", p=128))
        else:
            nc.gpsimd.dma_start(out=mb1c[:SH1, 0:1], in_=P["mb1s"][:].rearrange("(s o) -> s o", o=1))
        mb2c = single([128, nms4], "mb2c")
        nc.gpsimd.dma_start(out=mb2c, in_=P["mb2s"][:].rearrange("(m p) -> p m", p=128))
        fbc = single([128, L * nms4], "fbc")
        nc.gpsimd.dma_start(out=fbc, in_=P["fbs"][:, :].rearrange("l (m p) -> p (l m)", p=128))

        # residents that live through attention
        x0p = ec(tc.tile_pool(name="x0p_o", bufs=1))
        x0 = [x0p.tile([128, Q], BF16, name=f"x0{k}", tag=f"x0{k}") for k in range(nE)]
        # res-block weight pool opened early (before res_att, LIFO order):
        # its DMAs have no deps so the scheduler prefetches res weights
        # during attention
        rwp = ec(tc.tile_pool(name="rwp", bufs=1))
        res_att_cm = tc.tile_pool(name="res_att", bufs=1)
        res_att = res_att_cm.__enter__()
        kT = [res_att.tile([128, KVT * 128], BF16, name=f"kT{k}", tag=f"kT{k}") for k in range(nE)]
        ktmb = res_att.tile([128, nE], BF16, name="ktmb", tag="ktmb")
        va = [res_att.tile([128, H * 65], BF16, name=f"va{t}", tag=f"va{t}") for t in range(KVT)]
        va_t = res_att.tile([1, H * 65], BF16, name="va_t", tag="va_t")
        qT = [res_att.tile([128, Q], BF16, name=f"qT{k}", tag=f"qT{k}") for k in range(nE)]
        xat = [res_att.tile([128, Q], BF16, name=f"xat{k}", tag=f"xat{k}") for k in range(nE)]

        def layer_norm_block(dst_v, src_v, ntile, TBn, inv_col, pools):
            """dst = (src - mean) * rstd, feature-major; per-token stats."""
            mmp, statp, bcp, rowp = pools
            st_s = statp.tile([1, TBn], F32, name="st_s", tag="st_s")
            st_q = statp.tile([1, TBn], F32, name="st_q", tag="st_q")
            for k in range(ntile):
                if src_v[k].dtype == BF16:
                    xb = src_v[k]
                else:
                    xb = rowp.tile([128, TBn], BF16, name="ln_xb", tag="ln_xb",
                                   bufs=2)
                    nc.vector.tensor_copy(xb, src_v[k])
                sq = rowp.tile([128, TBn], BF16, name="ln_sq", tag="ln_sq", bufs=2)
                nc.vector.tensor_mul(sq, xb, xb)
                nc.tensor.matmul(st_s, _r(inv_col), _r(xb),
                                 start=(k == 0), stop=(k == ntile - 1))
                nc.tensor.matmul(st_q, _r(inv_col), _r(sq),
                                 start=(k == 0), stop=(k == ntile - 1))
            # rstd/mr feed matmul rhs -> must sit at base partition 0 like ones_row
            rstd = rowp.tile([1, TBn], BF16, name="ln_rstd", tag="ln_rstd", bufs=1)
            mr = rowp.tile([1, TBn], BF16, name="ln_mr", tag="ln_mr", bufs=1)
            mv = rowp.tile([33, TBn], F32, name="ln_mv", tag="ln_mv", bufs=1)
            mean_sb, var = mv[0:1], mv[32:33]
            # DVE has one PSUM read port: stage mean in SBUF first
            nc.vector.tensor_copy(mean_sb, st_s)
            msq = rowp.tile([1, TBn], F32, name="ln_msq", tag="ln_msq", bufs=1)
            nc.vector.tensor_mul(msq, mean_sb, mean_sb)
            nc.vector.tensor_sub(var, st_q, msq)
            nc.scalar.activation(out=rstd, in_=var, func=AF.Sqrt, bias=eps_row)
            with nc.allow_low_precision(reason="bf16 rstd feeds bf16 bcast matmul"):
                nc.vector.reciprocal(rstd, rstd)
            nc.vector.tensor_mul(mr, mean_sb, rstd)
            rstd_b = bcp.tile([128, TBn], F32, name="ln_rstd_b", tag="ln_rstd_b")
            mr_b = bcp.tile([128, TBn], F32, name="ln_mr_b", tag="ln_mr_b")
            nc.tensor.matmul(rstd_b, _r(ones_row), _r(rstd), start=True, stop=True)
            nc.tensor.matmul(mr_b, _r(ones_row), _r(mr), start=True, stop=True)
            for k in range(ntile):
                nc.vector.tensor_mul(dst_v[k], src_v[k], rstd_b)
                nc.vector.tensor_sub(dst_v[k], dst_v[k], mr_b)

        # ============ LN + film path + projections ============
        with tc.tile_pool(name="p1", bufs=3) as p1, \
             tc.tile_pool(name="sx", bufs=1) as sxp, \
             tc.tile_pool(name="mm1", bufs=3, space="PSUM") as mm1, \
             tc.tile_pool(name="stat1", bufs=1, space="PSUM") as stat1, \
             tc.tile_pool(name="bc1", bufs=1, space="PSUM") as bc1, \
             tc.tile_pool(name="fps", bufs=1, space="PSUM") as fps, \
             tc.tile_pool(name="rows1", bufs=4) as rows1:
            s1n = [sxp.tile([128, S], BF16, name=f"s1n{k}", tag=f"s1n{k}") for k in range(nE)]
            s2n = [sxp.tile([128, Q], BF16, name=f"s2n{k}", tag=f"s2n{k}") for k in range(nE)]

            # --- stream LN
            for sdram, ntok, dsts in ((P["s2T"], Q, s2n), (P["s1T"], S, s1n)):
                for blk in range(ntok // TB):
                    sl = slice(blk * TB, (blk + 1) * TB)
                    raw = [p1.tile([128, TB], F32, name="raw", tag="raw", bufs=nE + 3)
                           for _ in range(nE)]
                    for k in range(nE):
                        dma(out=raw[k], in_=sdram[k * 128 : (k + 1) * 128, sl])
                    layer_norm_block([d[:, sl] for d in dsts], raw, nE, TB, inv_E,
                                     (mm1, stat1, bc1, rows1))

            # --- film path (8-way sharded, 3 AllGathers)
            with tc.tile_pool(name="fwp", bufs=6) as fw_pool:
                for mi in range(nms1):
                    mm = min(128, SH1 - mi * 128)
                    wr = fw_pool.tile([128, nE * 128], BF16, name="fw1r",
                                      tag="fw1r", bufs=2)
                    nc.gpsimd.dma_start(out=wr, in_=P["mw1s"][mi].rearrange("p k c -> p (k c)"))
                    ps = fps.tile([128, B], F32, name="fps", tag="fps")
                    for k in range(nE):
                        nc.tensor.matmul(ps[:mm, :],
                                         _r(wr[:, k * 128 : k * 128 + mm]),
                                         _r(tembT_sb[:, k, :]),
                                         start=(k == 0), stop=(k == nE - 1))
                    hsb = p1.tile([128, B], BF16, name="fh", tag="fh")
                    bias_ap = (mb1c[:mm, mi : mi + 1] if SH1 % 128 == 0
                               else mb1c[mi * 128 : mi * 128 + mm, 0:1])
                    nc.scalar.activation(out=hsb[:mm, :], in_=ps[:mm, :],
                                         func=AF.Gelu, bias=bias_ap)
                    nc.gpsimd.dma_start(out=h_sh[mi * 128 : mi * 128 + mm, :], in_=hsb[:mm, :])
                nc.gpsimd.collective_compute(
                    "AllGather", ALU.bypass, replica_groups=[list(range(NC))],
                    ins=[h_sh[:, :]], outs=[h_g[:, :, :]])
                hfull = [p1.tile([128, B], BF16, name=f"hf{k}", tag=f"hf{k}", bufs=1)
                         for k in range(nD1)]
                for rr in range(NC):
                    off = 0
                    while off < SH1:
                        g = rr * SH1 + off
                        k, po = g // 128, g % 128
                        n = min(128 - po, SH1 - off)
                        nc.gpsimd.dma_start(out=hfull[k][po : po + n, :], in_=h_g[rr, off : off + n, :])
                        off += n
                for mi in range(nms4):
                    wr = fw_pool.tile([128, nD1 * 128], BF16, name="fwbig",
                                      tag="fwbig", bufs=1)
                    nc.gpsimd.dma_start(out=wr, in_=P["mw2s"][mi].rearrange("p k c -> p (k c)"))
                    ps = fps.tile([128, B], F32, name="fps", tag="fps")
                    for k in range(nD1):
                        nc.tensor.matmul(ps, _r(wr[:, k * 128 : (k + 1) * 128]),
                                         _r(hfull[k]),
                                         start=(k == 0), stop=(k == nD1 - 1))
                    tsb = p1.tile([128, B], BF16, name="ft", tag="ft")
                    nc.vector.tensor_scalar_add(out=tsb, in0=ps,
                                                scalar1=mb2c[:, mi : mi + 1])
                    nc.gpsimd.dma_start(out=t_sh[mi * 128 : (mi + 1) * 128, :], in_=tsb)
                nc.gpsimd.collective_compute(
                    "AllGather", ALU.bypass, replica_groups=[list(range(NC))],
                    ins=[t_sh[:, :]], outs=[t_g[:, :, :]])
                tfull = [p1.tile([128, B], BF16, name=f"tf{k}", tag=f"tf{k}", bufs=1)
                         for k in range(nD4)]
                for rr in range(NC):
                    for j in range(nms4):
                        g = rr * SH4 + j * 128
                        nc.gpsimd.dma_start(out=tfull[g // 128],
                            in_=t_g[rr, j * 128 : (j + 1) * 128, :])
                for l in range(L):
                    for mi in range(nms4):
                        wr = fw_pool.tile([128, nD4 * 128], BF16, name="fwbig",
                                          tag="fwbig", bufs=1)
                        nc.gpsimd.dma_start(out=wr,
                            in_=P["fws"][l, mi].rearrange("p k c -> p (k c)"))
                        ps = fps.tile([128, B], F32, name="fps", tag="fps")
                        for k in range(nD4):
                            nc.tensor.matmul(ps, _r(wr[:, k * 128 : (k + 1) * 128]),
                                             _r(tfull[k]),
                                             start=(k == 0), stop=(k == nD4 - 1))
                        fsb = p1.tile([128, B], BF16, name="ff", tag="ff")
                        nc.vector.tensor_scalar_add(
                            out=fsb, in0=ps,
                            scalar1=fbc[:, l * nms4 + mi : l * nms4 + mi + 1])
                        nc.gpsimd.dma_start(out=f_sh[l, mi * 128 : (mi + 1) * 128, :], in_=fsb)
                nc.gpsimd.collective_compute(
                    "AllGather", ALU.bypass, replica_groups=[list(range(NC))],
                    ins=[f_sh[:, :, :]], outs=[f_g[:, :, :, :]])

            # --- qT = wq1^T s1n[:, :Q] + wq2^T s2n + qb
            for m in range(nE):
                w1r = p1.tile([128, nE * 128], BF16, name="pw", tag="pw", bufs=4)
                dma(out=w1r, in_=P["wq1"][m].rearrange("p k c -> p (k c)"))
                w2r = p1.tile([128, nE * 128], BF16, name="pw", tag="pw", bufs=4)
                dma(out=w2r, in_=P["wq2"][m].rearrange("p k c -> p (k c)"))
                ps = mm1.tile([128, Q], F32, name="mm", tag="mm")
                for k in range(nE):
                    nc.tensor.matmul(ps, _r(w1r[:, k * 128 : (k + 1) * 128]),
                                     _r(s1n[k][:, 0:Q]),
                                     start=(k == 0), stop=False)
                for k in range(nE):
                    nc.tensor.matmul(ps, _r(w2r[:, k * 128 : (k + 1) * 128]),
                                     _r(s2n[k]), start=False, stop=(k == nE - 1))
                nc.vector.tensor_scalar_add(out=qT[m], in0=ps,
                                            scalar1=bcols["qb"][:, m : m + 1])

            # --- kT segments + ktmb
            for wname, bname, toks, src, ncol0 in (
                    ("wk2", "kb2", Q, s2n, nS1 * 128),
                    ("wk1", "kb1", S, s1n, 0)):
                for m in range(nE):
                    wr = p1.tile([128, nE * 128], BF16, name="pw", tag="pw", bufs=4)
                    dma(out=wr, in_=P[wname][m].rearrange("p k c -> p (k c)"))
                    for blk in range(toks // TB):
                        sl = slice(blk * TB, (blk + 1) * TB)
                        ps = mm1.tile([128, TB], F32, name="mm", tag="mm")
                        for k in range(nE):
                            nc.tensor.matmul(ps, _r(wr[:, k * 128 : (k + 1) * 128]),
                                             _r(src[k][:, sl]),
                                             start=(k == 0), stop=(k == nE - 1))
                        osl = slice(ncol0 + blk * TB, ncol0 + (blk + 1) * TB)
                        nc.vector.tensor_scalar_add(
                            out=kT[m][:, osl], in0=ps,
                            scalar1=bcols[bname][:, m : m + 1])
            for m in range(nE):
                wr = p1.tile([128, nE * 128], BF16, name="pw", tag="pw", bufs=4)
                dma(out=wr, in_=P["wkt"][m].rearrange("p k c -> p (k c)"))
                ps = mm1.tile([128, TB], F32, name="mm", tag="mm")
                for k in range(nE):
                    nc.tensor.matmul(ps[:, 0:1], _r(wr[:, k * 128 : (k + 1) * 128]),
                                     _r(temb_own_sb[:, k : k + 1]),
                                     start=(k == 0), stop=(k == nE - 1))
                nc.vector.tensor_scalar_add(out=ktmb[:, m : m + 1], in0=ps[:, 0:1],
                                            scalar1=bcols["kbt"][:, m : m + 1])

            # --- v (token-major, head-strided with ones column appended per head)
            for t in range(KVT):
                nc.vector.memset(
                    va[t][:, :].rearrange("p (h x) -> p h x", x=65)[:, :, 64:65], 1.0)
            nc.vector.memset(
                va_t[:, :].rearrange("p (h x) -> p h x", x=65)[:, :, 64:65], 1.0)
            for wname, vbn, tiles, src in (
                    ("wv2", "vb2", range(nS1, KVT), s2n),
                    ("wv1", "vb1", range(nS1), s1n)):
                wvr = []
                for k in range(nE):
                    wt = p1.tile([128, E], BF16, name="pwv", tag="pwv",
                                 bufs=nE + 3)
                    dma(out=wt, in_=P[wname][k * 128 : (k + 1) * 128, :])
                    wvr.append(wt)
                for c0 in range(0, E, 512):
                    cn = min(512, E - c0)
                    nh = cn // HD
                    h0 = c0 // HD
                    wts = [w[:, c0 : c0 + cn] for w in wvr]
                    for t in tiles:
                        lt = t if t < nS1 else t - nS1
                        tsl = slice(lt * 128, (lt + 1) * 128)
                        ps = mm1.tile([128, min(512, E)], F32, name="mm", tag="mm")
                        for k in range(nE):
                            nc.tensor.matmul(ps[:, :cn], _r(src[k][:, tsl]),
                                             _r(wts[k]),
                                             start=(k == 0), stop=(k == nE - 1))
                        nc.vector.tensor_tensor(
                            out=va[t][:, h0 * 65 : (h0 + nh) * 65].rearrange(
                                "p (h x) -> p h x", x=65)[:, :, 0:64],
                            in0=ps[:, :cn].rearrange("p (h x) -> p h x", x=64),
                            in1=vb_bc[vbn][:, c0 : c0 + cn].rearrange(
                                "p (h x) -> p h x", x=64),
                            op=ALU.add)
            wvr = []
            for k in range(nE):
                wt = p1.tile([128, E], BF16, name="pwv", tag="pwv", bufs=nE + 3)
                dma(out=wt, in_=P["wvt"][k * 128 : (k + 1) * 128, :])
                wvr.append(wt)
            for c0 in range(0, E, 512):
                cn = min(512, E - c0)
                nh = cn // HD
                h0 = c0 // HD
                ps = mm1.tile([128, min(512, E)], F32, name="mm", tag="mm")
                for k in range(nE):
                    nc.tensor.matmul(ps[0:1, :cn], _r(temb_own_sb[:, k : k + 1]),
                                     _r(wvr[k][:, c0 : c0 + cn]),
                                     start=(k == 0), stop=(k == nE - 1))
                nc.vector.tensor_tensor(
                    out=va_t[:, h0 * 65 : (h0 + nh) * 65].rearrange(
                        "p (h x) -> p h x", x=65)[:, :, 0:64],
                    in0=ps[0:1, :cn].rearrange("p (h x) -> p h x", x=64),
                    in1=vb_bc["vbt"][0:1, c0 : c0 + cn].rearrange(
                        "p (h x) -> p h x", x=64),
                    op=ALU.add)

        # ============ attention ============
        with tc.tile_pool(name="probs", bufs=8) as probs, \
             tc.tile_pool(name="arow", bufs=3) as arow, \
             tc.tile_pool(name="lg", bufs=3, space="PSUM") as lgp, \
             tc.tile_pool(name="avp", bufs=3, space="PSUM") as avp, \
             tc.tile_pool(name="rbp", bufs=2, space="PSUM") as rbp:
            # head PAIRS interleaved: the two K=64 logits matmuls sit at base
            # partitions 0/64 (disjoint PE row groups -> concurrent), and the
            # ACT exp of one head overlaps the PE work of the other.
            for hp in range(H // 2):
                mh = hp
                heads = (2 * hp, 2 * hp + 1)
                hsl = [slice(0, 64), slice(64, 128)]
                vsls = [slice(h * 65, (h + 1) * 65) for h in heads]
                avl = [avp.tile([128, Q], F32, name="av", tag="av")
                       for _ in range(2)]
                for j in range(2):
                    lg_t = lgp.tile([128, Q], F32, name="lg", tag="lg")
                    nc.tensor.matmul(lg_t[0:1, :], _r(ktmb[hsl[j], mh : mh + 1]),
                                     _r(qT[mh][hsl[j], :]), start=True, stop=True)
                    pr_t = arow.tile([1, Q], BF16, name="prt", tag="prt", bufs=4)
                    nc.scalar.activation(out=pr_t, in_=lg_t[0:1, :], func=AF.Exp)
                    nc.tensor.matmul(avl[j][0:65, :], _r(va_t[:, vsls[j]]),
                                     _r(pr_t), start=True, stop=False,
                                     skip_group_check=True)
                for t in range(KVT):
                    if t < nS1:
                        N = min(t + 1, nq) * 128
                        q0 = 0
                        dcol = t * 128 if t < nq else None
                        msk = m_s1
                    else:
                        q0 = (t - nS1) * 128
                        N = Q - q0
                        dcol = 0
                        msk = m_s2
                    prl = []
                    for j in range(2):
                        lg = lgp.tile([128, Q], F32, name="lg", tag="lg")
                        nc.tensor.matmul(lg[:, :N],
                                         _r(kT[mh][hsl[j], t * 128 : (t + 1) * 128]),
                                         _r(qT[mh][hsl[j], q0 : q0 + N]),
                                         start=True, stop=True)
                        pr = probs.tile([128, Q], BF16, name="pr", tag="pr")
                        nc.scalar.activation(out=pr[:, :N], in_=lg[:, :N],
                                             func=AF.Exp)
                        if dcol is not None:
                            nc.vector.tensor_mul(pr[:, dcol : dcol + 128],
                                                 pr[:, dcol : dcol + 128], msk)
                        prl.append(pr)
                    for j in range(2):
                        nc.tensor.matmul(avl[j][0:65, q0 : q0 + N],
                                         _r(va[t][:, vsls[j]]), _r(prl[j][:, :N]),
                                         start=False, stop=(t == KVT - 1),
                                         skip_group_check=True)
                for j in range(2):
                    rec = arow.tile([1, Q], BF16, name="rec", tag="rec", bufs=4)
                    with nc.allow_low_precision(reason="bf16 softmax norm"):
                        nc.vector.reciprocal(rec, avl[j][64:65, :])
                    rb = rbp.tile([128, Q], F32, name="rb", tag="rb")
                    nc.tensor.matmul(rb[0:HD, :], _r(ones_row[:, 0:HD]), _r(rec),
                                     start=True, stop=True)
                    avs = probs.tile([64, Q], F32, name="avs", tag="avs", bufs=3)
                    nc.vector.tensor_copy(avs, avl[j][0:HD, :])
                    nc.vector.tensor_tensor(out=xat[mh][hsl[j], :], in0=avs,
                                            in1=rb[0:HD, :], op=ALU.mult)
            # x0 = Wo^T xat + ob + (raw s1[:Q] + raw s2)   (inside attn scope)
            with tc.tile_pool(name="pwo", bufs=8) as pwo:
                for m in range(nE):
                    wr = pwo.tile([128, nE * 128], BF16, name="wow", tag="wow",
                                  bufs=3)
                    dma(out=wr, in_=P["wo"][m].rearrange("p k c -> p (k c)"))
                    ps = lgp.tile([128, Q], F32, name="lg", tag="lg")
                    for k in range(nE):
                        nc.tensor.matmul(ps, _r(wr[:, k * 128 : (k + 1) * 128]),
                                         _r(xat[k]),
                                         start=(k == 0), stop=(k == nE - 1))
                    r1 = pwo.tile([128, Q], F32, name="ir1", tag="ir1", bufs=2)
                    r2 = pwo.tile([128, Q], F32, name="ir2", tag="ir2", bufs=2)
                    dma(out=r1, in_=P["s1T"][m * 128 : (m + 1) * 128, 0:Q])
                    dma(out=r2, in_=P["s2T"][m * 128 : (m + 1) * 128, :])
                    nc.vector.tensor_add(r1, r1, r2)
                    nc.vector.scalar_tensor_tensor(
                        out=x0[m], in0=ps, scalar=bcols["ob"][:, m : m + 1],
                        in1=r1, op0=ALU.add, op1=ALU.add)

        res_att_cm.__exit__(None, None, None)

        # film select (batched): shard r of f_g holds film rows
        # [r*SH4,(r+1)*SH4); first NC/2 shards are `a`, rest are `b2`.
        # Runs on singles-pool tags so it overlaps attention/readout freely.
        for l in range(L):
            for r in range(NC):
                ft = single([128, nms4, B], f"fsel{l}_{r}", BF16)
                nc.gpsimd.dma_start(out=ft, in_=f_g[r, l].rearrange("(j p) b -> p j b", p=128))
                ohx = bass.AP(tensor=oh_b.tensor, offset=oh_b.offset,
                              ap=[[B, 128], [0, nms4], [1, B]])
                nc.vector.tensor_mul(ft, ft, ohx)
                dest = acol[l] if r < NC // 2 else b2col[l]
                c0 = (r % (NC // 2)) * nms4
                nc.vector.reduce_sum(out=dest[:, c0 : c0 + nms4], in_=ft,
                                     axis=mybir.AxisListType.X)

        # ============ readout ============
        with tc.tile_pool(name="p2", bufs=3) as p2, \
             tc.tile_pool(name="res2", bufs=1) as res2, \
             tc.tile_pool(name="mm2", bufs=4, space="PSUM") as mm2, \
             tc.tile_pool(name="stat2", bufs=1, space="PSUM") as stat2, \
             tc.tile_pool(name="bc2", bufs=1, space="PSUM") as bc2, \
             tc.tile_pool(name="rows2", bufs=4) as rows2:
            x1 = [res2.tile([128, Q], BF16, name=f"x1{k}", tag=f"x1{k}") for k in range(n2E)]
            hT = [res2.tile([128, Q], BF16, name=f"hT{k}", tag=f"hT{k}") for k in range(nD1)]

            # x1 = inw^T x0 + inb
            for m in range(n2E):
                wr = p2.tile([128, nE * 128], BF16, name="pw2", tag="pw2", bufs=4)
                dma(out=wr, in_=P["inw"][m].rearrange("p k c -> p (k c)"))
                ps = mm2.tile([128, Q], F32, name="mm", tag="mm")
                for k in range(nE):
                    nc.tensor.matmul(ps, _r(wr[:, k * 128 : (k + 1) * 128]),
                                     _r(x0[k]),
                                     start=(k == 0), stop=(k == nE - 1))
                nc.vector.tensor_scalar_add(out=x1[m], in0=ps,
                                            scalar1=bcols["inb"][:, m : m + 1])

            for l in range(L):
                for m in range(nD1):
                    wr = rwp.tile([128, n2E * 128], BF16, name="rw1r",
                                  tag="rw1r", bufs=11)
                    dma(out=wr, in_=P["rw1"][l, m].rearrange("p k c -> p (k c)"))
                    ps = mm2.tile([128, Q], F32, name="mm", tag="mm")
                    for k in range(n2E):
                        nc.tensor.matmul(
                            ps, _r(wr[:, k * 128 : (k + 1) * 128]),
                            _r(x1[k]), start=(k == 0), stop=(k == n2E - 1))
                    nc.scalar.activation(out=hT[m], in_=ps, func=AF.Gelu,
                                         bias=rb1c[l][:, m : m + 1])
                for m in range(n2E):
                    wr = rwp.tile([128, nD1 * 128], BF16, name="rw2r",
                                  tag="rw2r", bufs=4)
                    dma(out=wr, in_=P["rw2"][l, m].rearrange("p k c -> p (k c)"))
                    ps = mm2.tile([128, Q], F32, name="mm", tag="mm")
                    for k in range(nD1):
                        nc.tensor.matmul(
                            ps, _r(wr[:, k * 128 : (k + 1) * 128]),
                            _r(hT[k]), start=(k == 0), stop=(k == nD1 - 1))
                    # x1 <- x1 + z + rb2 (in place)
                    nc.vector.scalar_tensor_tensor(
                        out=x1[m], in0=ps, scalar=rb2c[l][:, m : m + 1],
                        in1=x1[m], op0=ALU.add, op1=ALU.add)
                layer_norm_block(x1, x1, n2E, Q, inv_2E, (mm2, stat2, bc2, rows2))
                for m in range(n2E):
                    nc.vector.tensor_scalar(out=x1[m], in0=x1[m],
                                            scalar1=acol[l][:, m : m + 1],
                                            scalar2=b2col[l][:, m : m + 1],
                                            op0=ALU.mult, op1=ALU.add)

            # logits out
            for m in range(nSO):
                mm = min(128, SO - m * 128)
                wr = p2.tile([128, n2E * 128], BF16, name="pw2l", tag="pw2l",
                             bufs=2)
                dma(out=wr, in_=P["logw"][m].rearrange("p k c -> p (k c)"))
                ps = mm2.tile([128, Q], F32, name="mm", tag="mm")
                for k in range(n2E):
                    nc.tensor.matmul(ps[:mm, :],
                                     _r(wr[:, k * 128 : k * 128 + mm]),
                                     _r(x1[k]),
                                     start=(k == 0), stop=(k == n2E - 1))
                osb = p2.tile([128, Q], F32, name="osb", tag="osb", bufs=2)
                nc.vector.tensor_scalar_add(out=osb[:mm, :], in0=ps[:mm, :],
                                            scalar1=lbc[:, m : m + 1])
                dma(out=outT[m * 128 : m * 128 + mm, :], in_=osb[:mm, :])

    nc.compile()
    return nc


# ---------------------------------------------------------------- entry point

_NC_CACHE = {}


LAST_RESULT = None


def _install_ntff_hook():
    """Register the axon NTFF profile hook that this image leaves unplugged."""
    import sys
    import types
    try:
        from antenv.axon_hooks import get_axon_ntff_profile_hook  # noqa: F401
        return
    except ImportError:
        pass
    import antenv
    m = types.ModuleType("antenv.axon_hooks")
    m._hook = None
    m.set_axon_ntff_profile_hook = lambda h: setattr(m, "_hook", h)
    m.get_axon_ntff_profile_hook = lambda: m._hook
    sys.modules["antenv.axon_hooks"] = m
    antenv.axon_hooks = m
    if "/root/.axon_site" not in sys.path:
        sys.path.insert(0, "/root/.axon_site")
    from trn_agent_boot.trn_boot import _ntff_profile_via_ctypes
    m._hook = _ntff_profile_via_ctypes("/opt/axon/libaxon_pjrt.so")
    from concourse import bass_utils as bu
    bu.upload_artifacts = lambda tmpdir: tmpdir


def kernel(**inputs):
    import os
    global LAST_RESULT
    cfg = REAL
    from concourse.bass_utils import run_bass_kernel_spmd
    maps = host_prep(cfg, inputs)
    if "real" not in _NC_CACHE:
        _NC_CACHE["real"] = build_nc(cfg)
    nc = _NC_CACHE["real"]
    trace = os.environ.get("KERNEL_TRACE", "") == "1"
    if trace:
        try:
            _install_ntff_hook()
        except Exception as e:
            print("ntff hook install failed:", e)
            trace = False
    res = run_bass_kernel_spmd(nc, maps, list(range(cfg.NC)), trace=trace,
                               tmpdir=os.environ.get("KERNEL_TRACE_DIR"))
    LAST_RESULT = res
    return host_gather(cfg, res.results)

